# revision 19
# baseline (speedup 1.0000x reference)
"""Bass kernel for nn_CHGAN (graph attention with edge-bias/decay softmax).

Device computes, per core (t-shard of 4 timesteps):
  q/k/v projections (per-node-type blockdiag weights, masked-x accumulation),
  row-tiled QK^T with PSUM bias/mask inject, DVE decay-multiply, ACT exp,
  col-tiled AV matmuls with fused Z (softmax denominator) row.
Host does: input prep (type masks, bias gather, decay) and the cheap
epilogue (normalize by Z, concat, output projection, residual).

Toolchain constraint honored throughout: one semaphore wait per instruction.
All buffers are persistent (no tile pools -> no slot-release waits); tiny
"absorber" reads keep every instruction at <=1 new cross-engine dependency;
total DMA count stays <=8 per DGE flavor (no queue wrap).
"""

import numpy as np

N, T, D, H, DH, NPATH, NTYPE, ETYPE = 512, 32, 64, 8, 8, 4, 4, 16
LAM = np.float32(0.5)
MASKC = np.float32(256.0)
NCORES = 8
TL = T // NCORES  # timesteps per core

_CACHE = {}


def build_bass():
    import concourse.bass as bass
    import concourse.mybir as mybir
    import concourse.tile as tile

    F32 = mybir.dt.float32
    BF16 = mybir.dt.bfloat16
    AF = mybir.ActivationFunctionType

    nc = bass.Bass()
    # ---- dram I/O (per core) ----
    xtg_d = nc.dram_tensor("xtg", (TL, 64, NTYPE * N), F32, kind="ExternalInput")
    cin_d = nc.dram_tensor("cin", (128, 16 * N + 160 + 2048 + 288), F32,
                           kind="ExternalInput")
    oraw_d = nc.dram_tensor("oraw", (TL, 2, 2, 128, N), F32, kind="ExternalOutput")

    # ---- persistent sbuf ----
    cin = nc.alloc_sbuf_tensor([128, 16 * N + 160 + 2048 + 288], F32)
    bdm = cin[:, 0:16 * N]
    cst = cin[:, 16 * N:16 * N + 160]
    wqk = cin[0:64, 16 * N + 160:16 * N + 160 + 2048]
    wv9 = cin[0:64, 16 * N + 160 + 2048:16 * N + 160 + 2048 + 288]
    xtgall = nc.alloc_sbuf_tensor([64, TL * NTYPE * N], F32)
    xtg2 = [xtgall[:, i * NTYPE * N:(i + 1) * NTYPE * N] for i in range(4)]
    qkTs = [nc.alloc_sbuf_tensor(f'qkTs{i}', [128, N], F32) for i in range(4)]
    vE = nc.alloc_sbuf_tensor([128, 4 * 72], F32)
    u_b = {(h, d): nc.alloc_sbuf_tensor(f'ub{h}{d}', [128, 4 * N], F32)
           for h in range(4) for d in range(2)}
    craw4 = nc.alloc_sbuf_tensor([128, TL * 4 * N], F32)
    scr_d = nc.alloc_sbuf_tensor([1, 4], F32)
    scr_a = nc.alloc_sbuf_tensor([1, 4], F32)

    # ---- persistent psum: 8 banks ----
    sbank = [nc.alloc_psum_tensor(f'sbank{i}', [128, N], F32) for i in range(4)]
    cbank = [nc.alloc_psum_tensor(f'cbank{i}', [128, N], F32) for i in range(4)]

    def wq_ap(g, half):
        c = (0 * NTYPE + g) * 256 + half * 128
        return wqk[:, c:c + 128]

    def wk_ap(g, half):
        c = (1 * NTYPE + g) * 256 + half * 128
        return wqk[:, c:c + 128]

    def bdm_ap(s, mi):  # s: 0=bf 1=dbf 2=df 3=dr
        c = (s * 4 + mi) * N
        return bdm[:, c:c + N]

    ident = cst[:, 0:128]
    ones_c = cst[:, 128:160]

    with tile.TileContext(nc) as tc:  # noqa: F841
        # input DMAs: cin + xtg on HWDGE, outs on HWDGE
        nc.sync.dma_start(cin[:], cin_d[:, :])
        xsrc = bass.AP(xtg_d, 0, [[NTYPE * N, 64], [64 * NTYPE * N, TL], [1, NTYPE * N]])
        nc.sync.dma_start(xtgall[:], xsrc)
        # start absorbers: advance each engine's clock past the input DMAs
        nc.tensor.ldweights(cin[0:1, 0:1].bitcast(BF16))
        nc.tensor.ldweights(xtgall[0:1, 0:1].bitcast(BF16))
        nc.vector.tensor_copy(scr_d[0:1, 0:1], bdm[0:1, 0:1])
        nc.scalar.copy(scr_a[0:1, 0:1], cst[0:1, 0:1])

        last_p = None
        last_cb = None

        for tl in range(TL):
            xtg = xtg2[tl]

            # ---- qkv ----
            for iW, wap in ((0, wq_ap), (1, wk_ap)):
                for half in range(2):
                    bank = sbank[2 * iW + half]
                    for g in range(NTYPE):
                        nc.tensor.matmul(
                            bank[:], wap(g, half), xtg[:, g * N:g * N + N],
                            start=(g == 0), stop=(g == NTYPE - 1))
            for mi in range(4):
                for g in range(NTYPE):
                    nc.tensor.matmul(
                        cbank[0][:, mi * 72:mi * 72 + 72],
                        xtg[:, g * N + mi * 128:g * N + mi * 128 + 128],
                        wv9[:, g * 72:g * 72 + 72],
                        start=(g == 0), stop=(g == NTYPE - 1))

            # psum -> sbuf copies (ACT); vE first so later ACT ticks cover it
            nc.scalar.copy(vE[:], cbank[0][:, 0:4 * 72])
            nc.scalar.copy(
                bass.AP(vE, 8, [[4 * 72, 128], [9, 32]]), ones_c[:, 0:32])
            for i in range(4):
                nc.scalar.copy(qkTs[i][:], sbank[i][:])

            for half in range(2):
                qT = qkTs[0 + half]
                kT = qkTs[2 + half]
                # DVE absorber: cover the newest ACT tick that the u-WAW /
                # bank-pair deps can reference (qkT copies for half 0, the
                # previous half's last exp for half 1)
                if half == 0:
                    nc.vector.tensor_copy(scr_d[0:1, 1:2], qkTs[3][0:1, 0:1])
                else:
                    nc.vector.tensor_copy(scr_d[0:1, 1:2], last_p[0:1, 0:1])
                for mi in range(4):
                    for h in range(4):
                        nc.tensor.matmul(sbank[h][:], ident, bdm_ap(0, mi),
                                         start=True, stop=False)
                    for h in range(4):
                        nc.tensor.matmul(
                            sbank[h][:],
                            kT[32 * h:32 * h + 8, mi * 128:mi * 128 + 128],
                            qT[32 * h:32 * h + 8, :],
                            start=False, stop=True, skip_group_check=True,
                            tile_position=(32 * h, 0))
                    for h in range(4):
                        nc.vector.tensor_mul(
                            u_b[(h, 0)][:, mi * N:mi * N + N],
                            sbank[h][:], bdm_ap(2, mi))
                    for h in range(4):
                        nc.tensor.matmul(sbank[h][:], ident, bdm_ap(1, mi),
                                         start=False, stop=True,
                                         skip_group_check=True)
                    for h in range(4):
                        nc.vector.tensor_mul(
                            u_b[(h, 1)][:, mi * N:mi * N + N],
                            sbank[h][:], bdm_ap(3, mi))
                # ACT absorber: cover PE AV ticks from previous (t,half)
                if last_cb is not None:
                    nc.scalar.copy(scr_a[0:1, 1:2], last_cb[0:1, 0:1])
                for dirn in range(2):
                    for h in range(4):
                        nc.scalar.activation(u_b[(h, dirn)][:],
                                             u_b[(h, dirn)][:], AF.Exp)
                for dirn in range(2):
                    cb = cbank[2 * dirn + half]
                    for mi in range(4):
                        for h in range(4):
                            col = mi * 72 + 9 * (4 * half + h)
                            nc.tensor.matmul(
                                cb[32 * h:32 * h + 9, :],
                                vE[:, col:col + 9],
                                u_b[(h, dirn)][:, mi * N:mi * N + N],
                                start=(mi == 0), stop=(mi == 3),
                                skip_group_check=True,
                                tile_position=(0, 32 * h))
                    last_cb = cb
                last_p = u_b[(3, 1)]

            # catT psum -> sbuf (ACT), then one out DMA.
            # Absorbers: cover each bank's AV PE tick (ascending tick order)
            # so the WAR wait from out-DMA(t-1) is the only new wait below.
            for b in (0, 2, 1, 3):
                nc.scalar.copy(scr_a[0:1, 2:3], cbank[b][0:1, 0:1])
            for dirn in range(2):
                for half in range(2):
                    c0 = tl * 4 * N + (2 * dirn + half) * N
                    nc.scalar.copy(craw4[:, c0:c0 + N],
                                   cbank[2 * dirn + half][:])

        # single output DMA for all timesteps
        dst = bass.AP(oraw_d, 0,
                      [[N, 128], [4 * 128 * N, TL], [2 * 128 * N, 2],
                       [128 * N, 2], [1, N]])
        nc.sync.dma_start(dst, craw4[:])

    _strip_same_engine_waits(nc)
    return nc


def _strip_same_engine_waits(nc):
    """Remove semantically redundant same-engine semaphore waits.

    The PSUM bank-pair tracker emits read-after-read ordering waits even
    between two instructions on the same serially-executing engine (DVE/ACT
    ops fully drain before the next issues, so program order already implies
    them).  walrus rejects instructions with more than one wait, so strip
    any wait on the engine's own semaphore (identified via its on_update
    entry) for the serial engines."""
    serial_prefix = ("DVE", "Activation", "Pool")
    # queues that carry ExternalOutput DMAs
    out_queues = set()
    for bi in nc.all_instructions():
        if type(bi).__name__ in ("InstDMACopy", "InstDMA"):
            outs = getattr(bi, "outs", None) or []
            refs = []
            for a in outs:
                r = getattr(a, "memref", None) or getattr(a, "memsetref", "")
                refs.append(str(r))
            if any("oraw" in r for r in refs):
                si = getattr(bi, "sync_info", None)
                if si is not None:
                    for u in (si.on_update or []):
                        if u.ant_name.startswith("DMA"):
                            out_queues.add(u.ant_name)
    for bi in nc.all_instructions():
        si = getattr(bi, "sync_info", None)
        if si is None:
            continue
        waits = si.on_wait
        if not waits:
            continue
        if type(bi).__name__ == "InstDrain" and len(waits) > 1:
            # Tail drain: engine completion is enforced by the all-engine
            # barrier that follows; input-DMA ticks are transitively implied
            # by engine work that consumed them. Only the output-DMA queue
            # needs an explicit wait (walrus allows very few waits here).
            keep = [w for w in waits if w.ant_name in out_queues]
            si.on_wait = keep
            continue
        own = {u.ant_name for u in (si.on_update or [])}
        keep = [w for w in waits
                if not (w.ant_name in own and w.ant_name.startswith(serial_prefix))]
        if len(keep) != len(waits):
            si.on_wait = keep


def _prep_inputs(inputs):
    nf = np.asarray(inputs["node_features"], np.float32)
    Q = np.asarray(inputs["Q"], np.float32)
    K = np.asarray(inputs["K"], np.float32)
    V = np.asarray(inputs["V"], np.float32)
    E = np.asarray(inputs["E"], np.float32)
    eb_w = np.asarray(inputs["eb_w"], np.float32)
    eb_b = np.asarray(inputs["eb_b"], np.float32)
    node_type = np.asarray(inputs["node_type"])

    xt = np.ascontiguousarray(nf.reshape(N, T, 64).transpose(2, 1, 0))  # (hd,t,n)

    tmask = np.stack([(node_type == g) for g in range(NTYPE)]).astype(np.float32)
    xtg_full = xt[:, :, None, :] * tmask[None, None, :, :]   # (hd, t, g, n)
    xtg_full = np.ascontiguousarray(xtg_full.transpose(1, 0, 2, 3))  # (t,hd,g,n)

    wqk = np.zeros((64, 2, NTYPE, 2, 128), np.float32)
    scl = np.float32(1.0 / np.sqrt(DH))
    for g in range(NTYPE):
        for h in range(H):
            half, hp = divmod(h, 4)
            for d in range(DH):
                wqk[h * 8 + d, 0, g, half, 32 * hp:32 * hp + 8] = Q[g, h, d] * scl
                wqk[h * 8 + d, 1, g, half, 32 * hp:32 * hp + 8] = K[g, h, d]
    wqk = np.ascontiguousarray(wqk.reshape(64, -1))
    wv9 = np.zeros((64, NTYPE, 72), np.float32)
    for g in range(NTYPE):
        for h in range(H):
            for d in range(DH):
                wv9[h * 8 + d, g, 9 * h:9 * h + 8] = V[g, h, d]
    wv9 = np.ascontiguousarray(wv9.reshape(64, -1))

    E_eff = E * (np.arange(ETYPE) != 0)[:, None]
    e_proj = E_eff @ eb_w[0]
    mats = {}
    for tag, sfx in (("f", ""), ("r", "_r")):
        ept = np.asarray(inputs["edge_path_type" + sfx])
        epl = np.asarray(inputs["edge_path_len" + sfx], np.float32)
        msk = np.asarray(inputs["mask" + sfx])
        bias = e_proj[ept].mean(axis=-1) + eb_b[0]            # (n, m)
        btil = bias.T - MASKC * (1.0 - msk.T)                 # (m, n)
        dec = np.exp(LAM * (epl - 1.0), dtype=np.float32).T   # (m, n)
        mats[tag] = (btil.astype(np.float32), dec.astype(np.float32))

    def mrows(a):  # (m=512, n) -> (128, 4, n)
        return a.reshape(4, 128, N).transpose(1, 0, 2)

    bdm = np.stack([mrows(mats["f"][0]), mrows(mats["r"][0] - mats["f"][0]),
                    mrows(mats["f"][1]), mrows(mats["r"][1])], axis=1)
    bdm = np.ascontiguousarray(bdm.reshape(128, 16 * N))

    cst = np.zeros((128, 160), np.float32)
    cst[:, 0:128] = np.eye(128, dtype=np.float32)
    cst[:, 128:160] = 1.0

    cin = np.zeros((128, 16 * N + 160 + 2048 + 288), np.float32)
    cin[:, 0:16 * N] = bdm
    cin[:, 16 * N:16 * N + 160] = cst
    cin[0:64, 16 * N + 160:16 * N + 160 + 2048] = wqk
    cin[0:64, 16 * N + 160 + 2048:] = wv9
    in_maps = []
    for c in range(NCORES):
        in_maps.append({
            "xtg": np.ascontiguousarray(
                xtg_full[c * TL:(c + 1) * TL].reshape(TL, 64, NTYPE * N)),
            "cin": cin,
        })
    return in_maps


def _postprocess(results, inputs):
    nf = np.asarray(inputs["node_features"], np.float32)
    out_w = np.asarray(inputs["out_w"], np.float32)
    out_b = np.asarray(inputs["out_b"], np.float32)
    num = np.empty((2, H, 8, T, N), np.float32)
    zden = np.empty((2, H, T, N), np.float32)
    for c in range(NCORES):
        oraw = results[c]["oraw"]  # (TL, dir, half, 128, N)
        for half in range(2):
            for hp in range(4):
                h = 4 * half + hp
                blk = oraw[:, :, half, 32 * hp:32 * hp + 9, :]  # (TL,2,9,N)
                num[:, h, :, c * TL:(c + 1) * TL, :] = \
                    blk[:, :, 0:8, :].transpose(1, 2, 0, 3)
                zden[:, h, c * TL:(c + 1) * TL, :] = \
                    blk[:, :, 8, :].transpose(1, 0, 2)
    att = num / zden[:, :, None]             # (dir, h, c, t, n)
    att = att.transpose(0, 4, 3, 1, 2)       # (dir, n, t, h, c)
    cat = np.concatenate([att[0].reshape(N, T, 64), att[1].reshape(N, T, 64)],
                         axis=-1)
    return (cat @ out_w.T + out_b + nf.reshape(N, T, 64)).astype(np.float32)


def kernel(**inputs):
    import time
    from concourse.bass_utils import run_bass_kernel_spmd
    if "nc" not in _CACHE:
        _CACHE["nc"] = build_bass()
    in_maps = _prep_inputs(inputs)
    t0 = time.time()
    try:
        res = run_bass_kernel_spmd(_CACHE["nc"], in_maps,
                                   core_ids=list(range(NCORES)))
    except Exception:
        # transient NRT wedges have been observed on this fabric; retry once
        time.sleep(2.0)
        res = run_bass_kernel_spmd(_CACHE["nc"], in_maps,
                                   core_ids=list(range(NCORES)))
    _CACHE["exec_wall_s"] = time.time() - t0
    _CACHE["last_result"] = res
    return _postprocess(res.results, inputs)
